# revision 20
# baseline (speedup 1.0000x reference)
"""Graph builder + host-side layout for the EdgeConv GNN kernel (parameterized).

Layout: dst-sharded edge-parallel. Nodes degree-sorted desc, dealt to NC cores
tile-by-tile. Per local tile q: NB[q]+1 blocks of 128 slots (last block = pad,
gathered but never aggregated). Slot (q,j,p) = j-th incoming edge of local node
q*128+p (src duplicated from edge 0 when j >= deg). Gather: one
dma_gather(transpose) per tile from B_full[BASE:] with idx = vslot - BASE int16.

Math: EdgeConv(m = relu([xi, xj-xi] @ W1 + b1) @ W2 + b2, max over dst) is
decomposed as A = x@(W1_top - W1_bot) + b1 (per-dst), B = x@W1_bot (per-src),
z = relu(A[dst] + B[src]), u = z @ W2, T[dst] = max u, out = T + b2.
"""

import numpy as np
import ml_dtypes

import concourse.bass as bass
import concourse.bacc as bacc
import concourse.tile as tile
from concourse import mybir

BF16 = ml_dtypes.bfloat16
P = 128
F32 = mybir.dt.float32
BF = mybir.dt.bfloat16
I16 = mybir.dt.int16
AX = mybir.AxisListType
ALU = mybir.AluOpType
ACT = mybir.ActivationFunctionType


class Cfg:
    def __init__(self, N, E, NC=8, base=32768, ch=448):
        self.N, self.E, self.NC, self.BASE, self.CH = N, E, NC, base, ch
        self.NTILES = (N + NC * P - 1) // (NC * P)     # local tiles per core
        self.NLOC = self.NTILES * P
        self.NVIRT = NC * self.NLOC
        self.NCH = (self.NLOC + ch - 1) // ch          # node-phase chunks
        assert self.NLOC % ch == 0 or True


# ---------------------------------------------------------------- host layout

def build_layout(cfg, edge_index):
    src = np.asarray(edge_index[0], dtype=np.int64)
    dst = np.asarray(edge_index[1], dtype=np.int64)
    N, NC, NLOC, NT = cfg.N, cfg.NC, cfg.NLOC, cfg.NTILES

    deg = np.bincount(dst, minlength=N)
    order = np.argsort(-deg, kind="stable")
    order_full = np.concatenate([order, np.full(cfg.NVIRT - N, -1, np.int64)])
    gt = order_full.reshape(cfg.NVIRT // P, P)
    local_nodes = np.full((NC, NLOC), -1, np.int64)
    for t in range(cfg.NVIRT // P):
        local_nodes[t % NC, (t // NC) * P:(t // NC + 1) * P] = gt[t]
    vslot = np.full(N, -1, np.int64)
    for c in range(NC):
        m = local_nodes[c] >= 0
        vslot[local_nodes[c][m]] = c * NLOC + np.nonzero(m)[0]
    assert (vslot >= 0).all()

    sorted_deg = np.concatenate([deg[order], np.zeros(cfg.NVIRT - N, np.int64)])
    tile_max = sorted_deg.reshape(cfg.NVIRT // P, P).max(axis=1)
    NB = np.array([max(1, int(tile_max[q * NC:(q + 1) * NC].max()))
                   for q in range(NT)])

    eorder = np.argsort(dst, kind="stable")
    csr = np.zeros(N + 1, np.int64)
    np.cumsum(deg, out=csr[1:])
    src_sorted = src[eorder]

    nslots = int((NB + 1).sum()) * P                   # incl. pad blocks
    grow = np.full((NC, nslots), cfg.NVIRT - 1, np.int64)  # default: pad row
    for c in range(NC):
        off = 0
        for q in range(NT):
            nb = int(NB[q])
            nodes = local_nodes[c, q * P:(q + 1) * P]
            for p in range(P):
                n = nodes[p]
                cols = off + np.arange(nb) * P + p
                if n < 0 or deg[n] == 0:
                    grow[c, cols] = 0
                else:
                    s = src_sorted[csr[n]:csr[n + 1]]
                    rows = vslot[s]
                    k = min(nb, len(s))
                    grow[c, cols[:k]] = rows[:k]
                    if nb > k:
                        grow[c, cols[k:]] = rows[0]
            off += (nb + 1) * P                        # skip pad block
    idx_bits = ((grow - cfg.BASE) % 65536).astype(np.uint16)
    ncols16 = nslots // 16
    wrapped = np.zeros((NC, P, ncols16), np.uint16)
    for c in range(NC):
        wrapped[c] = np.tile(idx_bits[c].reshape(ncols16, 16).T, (8, 1))
    return dict(deg=deg, local_nodes=local_nodes, vslot=vslot, NB=NB,
                nslots=nslots, idx_wrap=wrapped.view(np.int16))


def shard_inputs(cfg, inputs, lay):
    x = np.asarray(inputs["x"], np.float32)
    geo = np.asarray(inputs["geo"], np.float32)
    t = np.asarray(inputs["t"], np.float32)
    cat = np.asarray(inputs["category"]).astype(np.int64)
    NLOC = cfg.NLOC
    maps = []
    for c in range(cfg.NC):
        ln = lay["local_nodes"][c]
        m = ln >= 0
        idx = np.where(m, ln, 0)
        xg = np.concatenate([x, geo], axis=1)[idx] * m[:, None]
        tt = t[idx][:, 0] * m
        cc = np.where(m, cat[idx][:, 0], 0)
        oh = np.zeros((10, NLOC), np.float32)
        oh[cc, np.arange(NLOC)] = m.astype(np.float32)
        w = {
            "xg_T": np.ascontiguousarray(xg.T.astype(np.float32)),
            "t_row": np.ascontiguousarray(tt[None, :].astype(np.float32)),
            "cat_oh": oh,
            "idx_wrap": lay["idx_wrap"][c],
        }
        for k in ("w_i1", "w_i2", "w11", "w12", "w21", "w_sig"):
            w[k] = np.asarray(inputs[k], np.float32)
        w["w22"] = np.asarray(inputs["w22"], np.float32)
        w["emb_cat"] = np.asarray(inputs["emb_cat"], np.float32)
        for k in ("b_i1", "b_i2", "b11", "b12", "b21", "b22", "b_sig"):
            w[k] = np.asarray(inputs[k], np.float32).reshape(-1, 1)
        w["wg_col"] = np.asarray(inputs["wg"], np.float32).reshape(-1, 1)
        maps.append(w)
    return maps


def unshard_output(cfg, outs, lay):
    full = np.zeros((cfg.N, 4), np.float32)
    for c in range(cfg.NC):
        ln = lay["local_nodes"][c]
        m = ln >= 0
        full[ln[m]] = np.asarray(outs[c], np.float32)[:, m].T
    return full


# ---------------------------------------------------------------- graph


def build_graph(cfg, NB, nslots):
    NLOC, CH, NCH, NT = cfg.NLOC, cfg.CH, cfg.NCH, cfg.NTILES
    nc = bacc.Bacc(None, target_bir_lowering=False, debug=False,
                   num_swdge_queues=4)

    def param(name, shape, dt=F32, out=False):
        return nc.declare_dram_parameter(name, list(shape), dt, isOutput=out)

    xg = param("xg_T", (6, NLOC))
    t_row = param("t_row", (1, NLOC))
    cat_oh = param("cat_oh", (10, NLOC))
    idxp = param("idx_wrap", (P, nslots // 16), I16)
    w_i1 = param("w_i1", (6, P)); b_i1 = param("b_i1", (P, 1))
    w_i2 = param("w_i2", (P, P)); b_i2 = param("b_i2", (P, 1))
    w11 = param("w11", (4 * P, P)); b11 = param("b11", (P, 1))
    w12 = param("w12", (P, P)); b12 = param("b12", (P, 1))
    w21 = param("w21", (4 * P, P)); b21 = param("b21", (P, 1))
    w22 = param("w22", (P, 4)); b22 = param("b22", (4, 1))
    w_sig = param("w_sig", (64, 64)); b_sig = param("b_sig", (64, 1))
    emb = param("emb_cat", (10, 64))
    wg_col = param("wg_col", (32, 1))
    out = param("out", (4, NLOC), out=True)

    agin1 = nc.dram_tensor("agin1", [NLOC, P], BF)
    agin2 = nc.dram_tensor("agin2", [NLOC, P], BF)
    bfull1 = nc.dram_tensor("bfull1", [cfg.NVIRT, P], BF, addr_space="Shared")
    bfull2 = nc.dram_tensor("bfull2", [cfg.NVIRT, P], BF, addr_space="Shared")

    NBMAX = int(NB.max())
    LN2S = float(2.0 * np.log(25.0))

    with tile.TileContext(nc) as tc:
        with (
            tc.tile_pool(name="pers", bufs=1) as pers,
            tc.tile_pool(name="wk", bufs=4) as wk,
            tc.tile_pool(name="gp", bufs=3) as gp,
            tc.tile_pool(name="ps", bufs=2, space="PSUM") as ps,
            tc.tile_pool(name="ps2", bufs=2, space="PSUM") as ps2,
        ):
            # ---- small weights/consts (persistent)
            def load(pp, shape, dt=F32):
                tl = pers.tile(list(shape), dt, tag=pp.name)
                nc.sync.dma_start(out=tl[:], in_=pp[:])
                return tl

            idxt = load(idxp, (P, nslots // 16), I16)
            wi1 = load(w_i1, (6, P))
            bi1 = load(b_i1, (P, 1)); bi2 = load(b_i2, (P, 1))
            b11t = load(b11, (P, 1)); b12t = load(b12, (P, 1))
            b21t = load(b21, (P, 1)); b22t = load(b22, (4, 1))
            bsig = load(b_sig, (64, 1))
            embt = load(emb, (10, 64))
            wgc = load(wg_col, (32, 1))

            def cast_load(pp, shape, tag):
                f = wk.tile(list(shape), F32, tag=f"{tag}_f")
                nc.sync.dma_start(out=f[:], in_=pp[:])
                b = pers.tile(list(shape), BF, tag=tag)
                nc.vector.tensor_copy(out=b[:], in_=f[:])
                return b

            wi2b = cast_load(w_i2, (P, P), "wi2b")
            wsigb = cast_load(w_sig, (64, 64), "wsigb")
            w12b = cast_load(w12, (P, P), "w12b")
            w22b = cast_load(w22, (P, 4), "w22b")

            def conv_w(wp, tag):
                parts = []
                for i in range(4):
                    tl = wk.tile([P, P], F32, tag=f"wld{i}")
                    nc.sync.dma_start(out=tl[:], in_=wp[i * P:(i + 1) * P, :])
                    parts.append(tl)
                wda = pers.tile([P, P], BF, tag=f"{tag}da")
                wdb = pers.tile([P, P], BF, tag=f"{tag}db")
                wbh = pers.tile([P, P], BF, tag=f"{tag}bh")
                wbc = pers.tile([P, P], BF, tag=f"{tag}bc")
                nc.vector.tensor_tensor(out=wda[:], in0=parts[0][:],
                                        in1=parts[2][:], op=ALU.subtract)
                nc.vector.tensor_tensor(out=wdb[:], in0=parts[1][:],
                                        in1=parts[3][:], op=ALU.subtract)
                nc.vector.tensor_copy(out=wbh[:], in_=parts[2][:])
                nc.vector.tensor_copy(out=wbc[:], in_=parts[3][:])
                return wda, wdb, wbh, wbc

            w1da, w1db, w1bh, w1bc = conv_w(w11, "w1")
            w2da, w2db, w2bh, w2bc = conv_w(w21, "w2")

            ones32 = pers.tile([1, 32], F32, tag="ones32")
            nc.vector.memset(ones32[:], 1.0)
            ones4 = pers.tile([1, 4], F32, tag="ones4")
            nc.vector.memset(ones4[:], 1.0)
            wg2pi = pers.tile([32, 1], F32, tag="wg2pi")
            nc.scalar.mul(out=wg2pi[:], in_=wgc[:], mul=float(2 * np.pi))
            negl = pers.tile([1, 1], F32, tag="negl")
            nc.vector.memset(negl[:], float(-1.0 / LN2S))
            negpi = pers.tile([32, 1], F32, tag="negpi")
            nc.vector.memset(negpi[:], float(-np.pi))

            # ---- persistent activations (bf16, [*, NLOC])
            h0 = pers.tile([P, NLOC], BF, tag="h0")
            cond = pers.tile([P, NLOC], BF, tag="cond")
            At = pers.tile([P, NLOC], BF, tag="At")
            T1 = pers.tile([P, NLOC], BF, tag="T1")
            T2 = pers.tile([4, NLOC], BF, tag="T2")

            # ---- node phase (per-chunk streaming)
            for i in range(NCH):
                s = slice(i * CH, (i + 1) * CH)
                xgc = wk.tile([6, CH], F32, tag="xgc")
                nc.sync.dma_start(out=xgc[:], in_=xg[:, s])
                pt = ps.tile([P, CH], F32, tag="pmm")
                nc.tensor.matmul(pt[:], wi1[:], xgc[:], start=True, stop=True)
                h0a = wk.tile([P, CH], BF, tag="h0a")
                nc.scalar.activation(out=h0a[:], in_=pt[:], func=ACT.Relu,
                                     bias=bi1[:])
                pt2 = ps.tile([P, CH], F32, tag="pmm")
                nc.tensor.matmul(pt2[:], wi2b[:], h0a[:], start=True, stop=True)
                nc.scalar.activation(out=h0[:, s], in_=pt2[:], func=ACT.Relu,
                                     bias=bi2[:])

                ohc = wk.tile([10, CH], F32, tag="ohc")
                nc.sync.dma_start(out=ohc[:], in_=cat_oh[:, s])
                pc = ps.tile([64, CH], F32, tag="pmm")
                nc.tensor.matmul(pc[:], embt[:], ohc[:], start=True, stop=True)
                nc.scalar.activation(out=cond[0:64, s], in_=pc[:], func=ACT.Relu)

                trc = wk.tile([1, CH], F32, tag="trc")
                nc.sync.dma_start(out=trc[:], in_=t_row[:, s])
                ptt = ps.tile([32, CH], F32, tag="pmm")
                nc.tensor.matmul(ptt[:], ones32[:], trc[:], start=True, stop=True)
                MAGIC = 12582912.0  # 1.5*2^23: f32 add/sub rounds to int
                tp = wk.tile([32, CH], F32, tag="tp")
                nc.vector.tensor_scalar(out=tp[:], in0=ptt[:], scalar1=wg2pi[:],
                                        scalar2=None, op0=ALU.mult)
                gfc = wk.tile([64, CH], BF, tag="gfc")
                for half, phase in ((0, 0.0), (1, 0.25)):
                    v = wk.tile([32, CH], F32, tag="yr")
                    if phase == 0.0:
                        nc.vector.tensor_scalar(out=v[:], in0=ptt[:],
                                                scalar1=wgc[:], scalar2=MAGIC,
                                                op0=ALU.mult, op1=ALU.add)
                    else:
                        nc.vector.tensor_scalar(out=v[:], in0=ptt[:],
                                                scalar1=wgc[:],
                                                scalar2=float(phase),
                                                op0=ALU.mult, op1=ALU.add)
                        nc.vector.tensor_scalar(out=v[:], in0=v[:],
                                                scalar1=MAGIC, scalar2=None,
                                                op0=ALU.add)
                    m = wk.tile([32, CH], F32, tag="mr")
                    nc.vector.tensor_scalar(out=m[:], in0=v[:], scalar1=MAGIC,
                                            scalar2=None, op0=ALU.subtract)
                    nc.vector.tensor_scalar(
                        out=m[:], in0=m[:], scalar1=float(-2 * np.pi),
                        scalar2=float(phase * 2 * np.pi),
                        op0=ALU.mult, op1=ALU.add)
                    r = wk.tile([32, CH], F32, tag="rr")
                    nc.vector.tensor_tensor(out=r[:], in0=tp[:], in1=m[:],
                                            op=ALU.add)
                    nc.scalar.activation(out=gfc[half * 32:(half + 1) * 32, :],
                                         in_=r[:], func=ACT.Sin)
                pg = ps.tile([64, CH], F32, tag="pmm")
                nc.tensor.matmul(pg[:], wsigb[:], gfc[:], start=True, stop=True)
                nc.scalar.activation(out=cond[64:128, s], in_=pg[:],
                                     func=ACT.Relu, bias=bsig[:])

            def make_AB(hsrc, wda, wdb, wbh, wbc, bA, agin):
                for i in range(NCH):
                    s = slice(i * CH, (i + 1) * CH)
                    pt = ps.tile([P, CH], F32, tag="pmm")
                    nc.tensor.matmul(pt[:], wda[:], hsrc[:, s], start=True,
                                     stop=False)
                    nc.tensor.matmul(pt[:], wdb[:], cond[:, s], start=False,
                                     stop=True)
                    nc.scalar.activation(out=At[:, s], in_=pt[:],
                                         func=ACT.Identity, bias=bA[:])
                for q in range(NT):
                    s = slice(q * P, (q + 1) * P)
                    pb = ps.tile([P, P], F32, tag="pbr")
                    nc.tensor.matmul(pb[:], hsrc[:, s], wbh[:], start=True,
                                     stop=False)
                    nc.tensor.matmul(pb[:], cond[:, s], wbc[:], start=False,
                                     stop=True)
                    rows = wk.tile([P, P], BF, tag="brows")
                    nc.scalar.activation(out=rows[:], in_=pb[:], func=ACT.Copy)
                    nc.sync.dma_start(out=agin[s, :], in_=rows[:])

            def edge_conv(bfull, wout, outp, T):
                off16 = 0
                for q in range(NT):
                    nb = int(NB[q])
                    ncols = (nb + 1) * P
                    g = gp.tile([P, 1, (NBMAX + 1) * P], BF, tag="G")
                    nc.gpsimd.dma_gather(
                        g[:, :, :ncols], bfull[cfg.BASE:, :],
                        idxt[:, off16:off16 + ncols // 16],
                        ncols, ncols, P, transpose=True,
                        single_packet=False, queue_num=0)
                    qs = slice(q * P, (q + 1) * P)
                    for j0 in range(0, nb, 4):
                        jc = min(4, nb - j0)
                        gsl = g[:, 0, j0 * P:(j0 + jc) * P]
                        zt = wk.tile([P, 4 * P], BF, tag="zt")
                        ab = At[:, qs].unsqueeze(1).broadcast_to((P, jc, P))
                        nc.vector.tensor_tensor(
                            out=zt[:, :jc * P].rearrange("p (j c) -> p j c", c=P),
                            in0=gsl.rearrange("p (j c) -> p j c", c=P),
                            in1=ab, op=ALU.add)
                        nc.scalar.activation(out=zt[:, :jc * P],
                                             in_=zt[:, :jc * P], func=ACT.Relu)
                        pu = ps2.tile([outp, 4 * P], F32, tag="pu")
                        nc.tensor.matmul(pu[:, :jc * P], wout[:], zt[:, :jc * P],
                                         start=True, stop=True)
                        red = wk.tile([outp, P], BF, tag="red")
                        nc.vector.tensor_reduce(
                            out=red[:], in_=pu[:, :jc * P].rearrange(
                                "p (j c) -> p c j", c=P),
                            axis=AX.X, op=ALU.max)
                        nc.vector.tensor_tensor(out=T[:, qs], in0=T[:, qs],
                                                in1=red[:], op=ALU.max)
                    off16 += ncols // 16

            # ---- conv1
            nc.vector.memset(T1[:], -1e30)
            make_AB(h0, w1da, w1db, w1bh, w1bc, b11t, agin1)
            nc.gpsimd.collective_compute(
                "AllGather", ALU.bypass,
                replica_groups=[list(range(cfg.NC))],
                ins=[agin1.ap().opt()], outs=[bfull1.ap().opt()])
            edge_conv(bfull1, w12b, P, T1)

            # ---- h1 = relu(T1 + b12) into h0 buffer; conv2
            for i in range(NCH):
                s = slice(i * CH, (i + 1) * CH)
                nc.scalar.activation(out=h0[:, s], in_=T1[:, s], func=ACT.Relu,
                                     bias=b12t[:])
            nc.vector.memset(T2[:], -1e30)
            make_AB(h0, w2da, w2db, w2bh, w2bc, b21t, agin2)
            nc.gpsimd.collective_compute(
                "AllGather", ALU.bypass,
                replica_groups=[list(range(cfg.NC))],
                ins=[agin2.ap().opt()], outs=[bfull2.ap().opt()])
            edge_conv(bfull2, w22b, 4, T2)

            # ---- epilogue per chunk: out = (T2 + b22) / (std + 1e-7)
            for i in range(NCH):
                s = slice(i * CH, (i + 1) * CH)
                trc = wk.tile([1, CH], F32, tag="trc")
                nc.sync.dma_start(out=trc[:], in_=t_row[:, s])
                sr = wk.tile([1, CH], F32, tag="sr")
                nc.scalar.activation(out=sr[:], in_=trc[:], func=ACT.Exp,
                                     scale=LN2S)
                nc.scalar.activation(out=sr[:], in_=sr[:], func=ACT.Sqrt,
                                     scale=float(1.0 / LN2S), bias=negl[:])
                nc.vector.tensor_scalar_add(out=sr[:], in0=sr[:], scalar1=1e-7)
                iv = wk.tile([1, CH], F32, tag="iv")
                nc.vector.reciprocal(out=iv[:], in_=sr[:])
                pt4 = ps.tile([4, CH], F32, tag="pmm")
                nc.tensor.matmul(pt4[:], ones4[:], iv[:], start=True, stop=True)
                osb = wk.tile([4, CH], F32, tag="osb")
                nc.scalar.activation(out=osb[:], in_=T2[:, s],
                                     func=ACT.Identity, bias=b22t[:])
                nc.vector.tensor_tensor(out=osb[:], in0=osb[:], in1=pt4[:],
                                        op=ALU.mult)
                nc.sync.dma_start(out=out[:, s], in_=osb[:])
    return nc


# ----------------------------------------------------------------- entry point

_CACHE = {}


def kernel(**inputs):
    """Self-contained Trainium2 kernel: takes FULL inputs, returns FULL [N,4]."""
    from concourse.bass_utils import run_bass_kernel_spmd

    cfg = Cfg(50000, 800000, base=32768, ch=448)
    edge_key = None
    lay = build_layout(cfg, np.asarray(inputs["edge_index"]))
    maps = shard_inputs(cfg, inputs, lay)
    nc = build_graph(cfg, lay["NB"], lay["nslots"])
    nc.compile()
    res = run_bass_kernel_spmd(nc, maps, list(range(cfg.NC)))
    outs = [res.results[c]["out"] for c in range(cfg.NC)]
    return unshard_output(cfg, outs, lay)



# revision 33
# speedup vs baseline: 1.0028x; 1.0028x over previous
"""Graph builder + host-side layout for the EdgeConv GNN kernel (parameterized).

Layout: dst-sharded edge-parallel. Nodes degree-sorted desc, dealt to NC cores
tile-by-tile. Per local tile q: NB[q]+1 blocks of 128 slots (last block = pad,
gathered but never aggregated). Slot (q,j,p) = j-th incoming edge of local node
q*128+p (src duplicated from edge 0 when j >= deg). Gather: one
dma_gather(transpose) per tile from B_full[BASE:] with idx = vslot - BASE int16.

Math: EdgeConv(m = relu([xi, xj-xi] @ W1 + b1) @ W2 + b2, max over dst) is
decomposed as A = x@(W1_top - W1_bot) + b1 (per-dst), B = x@W1_bot (per-src),
z = relu(A[dst] + B[src]), u = z @ W2, T[dst] = max u, out = T + b2.
"""

import numpy as np
import ml_dtypes

import concourse.bass as bass
import concourse.bacc as bacc
import concourse.tile as tile
from concourse import mybir

BF16 = ml_dtypes.bfloat16
P = 128
F32 = mybir.dt.float32
BF = mybir.dt.bfloat16
I16 = mybir.dt.int16
AX = mybir.AxisListType
ALU = mybir.AluOpType
ACT = mybir.ActivationFunctionType


class Cfg:
    def __init__(self, N, E, NC=8, base=32768, ch=448):
        self.N, self.E, self.NC, self.BASE, self.CH = N, E, NC, base, ch
        self.NTILES = (N + NC * P - 1) // (NC * P)     # local tiles per core
        self.NLOC = self.NTILES * P
        self.NVIRT = NC * self.NLOC
        self.NCH = (self.NLOC + ch - 1) // ch          # node-phase chunks
        assert self.NLOC % ch == 0 or True


# ---------------------------------------------------------------- host layout

def build_layout(cfg, edge_index):
    src = np.asarray(edge_index[0], dtype=np.int64)
    dst = np.asarray(edge_index[1], dtype=np.int64)
    N, NC, NLOC, NT = cfg.N, cfg.NC, cfg.NLOC, cfg.NTILES

    deg = np.bincount(dst, minlength=N)
    order = np.argsort(-deg, kind="stable")
    order_full = np.concatenate([order, np.full(cfg.NVIRT - N, -1, np.int64)])
    gt = order_full.reshape(cfg.NVIRT // P, P)
    local_nodes = np.full((NC, NLOC), -1, np.int64)
    for t in range(cfg.NVIRT // P):
        local_nodes[t % NC, (t // NC) * P:(t // NC + 1) * P] = gt[t]
    vslot = np.full(N, -1, np.int64)
    for c in range(NC):
        m = local_nodes[c] >= 0
        vslot[local_nodes[c][m]] = c * NLOC + np.nonzero(m)[0]
    assert (vslot >= 0).all()

    sorted_deg = np.concatenate([deg[order], np.zeros(cfg.NVIRT - N, np.int64)])
    tile_max = sorted_deg.reshape(cfg.NVIRT // P, P).max(axis=1)
    NB = np.array([max(1, int(tile_max[q * NC:(q + 1) * NC].max()))
                   for q in range(NT)])

    eorder = np.argsort(dst, kind="stable")
    csr = np.zeros(N + 1, np.int64)
    np.cumsum(deg, out=csr[1:])
    src_sorted = src[eorder]

    nslots = int((NB + 1).sum()) * P                   # incl. pad blocks
    grow = np.full((NC, nslots), cfg.NVIRT - 1, np.int64)  # default: pad row
    for c in range(NC):
        off = 0
        for q in range(NT):
            nb = int(NB[q])
            nodes = local_nodes[c, q * P:(q + 1) * P]
            for p in range(P):
                n = nodes[p]
                cols = off + np.arange(nb) * P + p
                if n < 0 or deg[n] == 0:
                    grow[c, cols] = 0
                else:
                    s = src_sorted[csr[n]:csr[n + 1]]
                    rows = vslot[s]
                    k = min(nb, len(s))
                    grow[c, cols[:k]] = rows[:k]
                    if nb > k:
                        grow[c, cols[k:]] = rows[0]
            off += (nb + 1) * P                        # skip pad block
    idx_bits = ((grow - cfg.BASE) % 65536).astype(np.uint16)
    ncols16 = nslots // 16
    wrapped = np.zeros((NC, P, ncols16), np.uint16)
    for c in range(NC):
        wrapped[c] = np.tile(idx_bits[c].reshape(ncols16, 16).T, (8, 1))
    return dict(deg=deg, local_nodes=local_nodes, vslot=vslot, NB=NB,
                nslots=nslots, idx_wrap=wrapped.view(np.int16))


def shard_inputs(cfg, inputs, lay):
    x = np.asarray(inputs["x"], np.float32)
    geo = np.asarray(inputs["geo"], np.float32)
    t = np.asarray(inputs["t"], np.float32)
    cat = np.asarray(inputs["category"]).astype(np.int64)
    NLOC = cfg.NLOC
    maps = []
    for c in range(cfg.NC):
        ln = lay["local_nodes"][c]
        m = ln >= 0
        idx = np.where(m, ln, 0)
        xg = np.concatenate([x, geo], axis=1)[idx] * m[:, None]
        tt = t[idx][:, 0] * m
        cc = np.where(m, cat[idx][:, 0], 0)
        oh = np.zeros((10, NLOC), np.float32)
        oh[cc, np.arange(NLOC)] = m.astype(np.float32)
        w = {
            "xg_T": np.ascontiguousarray(xg.T.astype(np.float32)),
            "t_row": np.ascontiguousarray(tt[None, :].astype(np.float32)),
            "cat_oh": oh,
            "idx_wrap": lay["idx_wrap"][c],
        }
        for k in ("w_i1", "w_i2", "w11", "w12", "w21", "w_sig"):
            w[k] = np.asarray(inputs[k], np.float32)
        w["w22"] = np.asarray(inputs["w22"], np.float32)
        w["emb_cat"] = np.asarray(inputs["emb_cat"], np.float32)
        for k in ("b_i1", "b_i2", "b11", "b12", "b21", "b22", "b_sig"):
            w[k] = np.asarray(inputs[k], np.float32).reshape(-1, 1)
        w["wg_col"] = np.asarray(inputs["wg"], np.float32).reshape(-1, 1)
        maps.append(w)
    return maps


def unshard_output(cfg, outs, lay):
    full = np.zeros((cfg.N, 4), np.float32)
    for c in range(cfg.NC):
        ln = lay["local_nodes"][c]
        m = ln >= 0
        full[ln[m]] = np.asarray(outs[c], np.float32)[:, m].T
    return full


# ---------------------------------------------------------------- graph


def build_graph(cfg, NB, nslots):
    NLOC, CH, NCH, NT = cfg.NLOC, cfg.CH, cfg.NCH, cfg.NTILES
    nc = bacc.Bacc(None, target_bir_lowering=False, debug=False,
                   num_swdge_queues=1, dynamic_dma_scratch_size=32768)

    def param(name, shape, dt=F32, out=False):
        return nc.declare_dram_parameter(name, list(shape), dt, isOutput=out)

    xg = param("xg_T", (6, NLOC))
    t_row = param("t_row", (1, NLOC))
    cat_oh = param("cat_oh", (10, NLOC))
    idxp = param("idx_wrap", (P, nslots // 16), I16)
    w_i1 = param("w_i1", (6, P)); b_i1 = param("b_i1", (P, 1))
    w_i2 = param("w_i2", (P, P)); b_i2 = param("b_i2", (P, 1))
    w11 = param("w11", (4 * P, P)); b11 = param("b11", (P, 1))
    w12 = param("w12", (P, P)); b12 = param("b12", (P, 1))
    w21 = param("w21", (4 * P, P)); b21 = param("b21", (P, 1))
    w22 = param("w22", (P, 4)); b22 = param("b22", (4, 1))
    w_sig = param("w_sig", (64, 64)); b_sig = param("b_sig", (64, 1))
    emb = param("emb_cat", (10, 64))
    wg_col = param("wg_col", (32, 1))
    out = param("out", (4, NLOC), out=True)

    agin1 = nc.dram_tensor("agin1", [NLOC, P], BF)
    agin2 = nc.dram_tensor("agin2", [NLOC, P], BF)
    bfull1 = nc.dram_tensor("bfull1", [cfg.NVIRT, P], BF, addr_space="Shared")
    bfull2 = nc.dram_tensor("bfull2", [cfg.NVIRT, P], BF, addr_space="Shared")

    NBMAX = int(NB.max())
    LN2S = float(2.0 * np.log(25.0))

    with tile.TileContext(nc) as tc:
        with (
            tc.tile_pool(name="pers", bufs=1) as pers,
            tc.tile_pool(name="wk", bufs=3) as wk,
            tc.tile_pool(name="gp", bufs=3) as gp,
            tc.tile_pool(name="ps", bufs=2, space="PSUM") as ps,
            tc.tile_pool(name="ps2", bufs=2, space="PSUM") as ps2,
        ):
            # ---- small weights/consts (persistent)
            def load(pp, shape, dt=F32):
                tl = pers.tile(list(shape), dt, tag=pp.name)
                nc.sync.dma_start(out=tl[:], in_=pp[:])
                return tl

            idxt = load(idxp, (P, nslots // 16), I16)
            wi1 = load(w_i1, (6, P))
            bi1 = load(b_i1, (P, 1)); bi2 = load(b_i2, (P, 1))
            b11t = load(b11, (P, 1)); b12t = load(b12, (P, 1))
            b21t = load(b21, (P, 1)); b22t = load(b22, (4, 1))
            bsig = load(b_sig, (64, 1))
            embt = load(emb, (10, 64))
            wgc = load(wg_col, (32, 1))

            def cast_load(pp, shape, tag):
                f = wk.tile(list(shape), F32, tag=f"{tag}_f")
                nc.sync.dma_start(out=f[:], in_=pp[:])
                b = pers.tile(list(shape), BF, tag=tag)
                nc.vector.tensor_copy(out=b[:], in_=f[:])
                return b

            wi2b = cast_load(w_i2, (P, P), "wi2b")
            wsigb = cast_load(w_sig, (64, 64), "wsigb")
            w12b = cast_load(w12, (P, P), "w12b")
            w22b = cast_load(w22, (P, 4), "w22b")

            def conv_w(wp, tag):
                parts = []
                for i in range(4):
                    tl = wk.tile([P, P], F32, tag=f"wld{i}")
                    nc.sync.dma_start(out=tl[:], in_=wp[i * P:(i + 1) * P, :])
                    parts.append(tl)
                wda = pers.tile([P, P], BF, tag=f"{tag}da")
                wdb = pers.tile([P, P], BF, tag=f"{tag}db")
                wbh = pers.tile([P, P], BF, tag=f"{tag}bh")
                wbc = pers.tile([P, P], BF, tag=f"{tag}bc")
                nc.vector.tensor_tensor(out=wda[:], in0=parts[0][:],
                                        in1=parts[2][:], op=ALU.subtract)
                nc.vector.tensor_tensor(out=wdb[:], in0=parts[1][:],
                                        in1=parts[3][:], op=ALU.subtract)
                nc.vector.tensor_copy(out=wbh[:], in_=parts[2][:])
                nc.vector.tensor_copy(out=wbc[:], in_=parts[3][:])
                return wda, wdb, wbh, wbc

            w1da, w1db, w1bh, w1bc = conv_w(w11, "w1")
            w2da, w2db, w2bh, w2bc = conv_w(w21, "w2")

            ones32 = pers.tile([1, 32], F32, tag="ones32")
            nc.vector.memset(ones32[:], 1.0)
            ones4 = pers.tile([1, 4], F32, tag="ones4")
            nc.vector.memset(ones4[:], 1.0)
            wg2pi = pers.tile([32, 1], F32, tag="wg2pi")
            nc.scalar.mul(out=wg2pi[:], in_=wgc[:], mul=float(2 * np.pi))
            negl = pers.tile([1, 1], F32, tag="negl")
            nc.vector.memset(negl[:], float(-1.0 / LN2S))
            negpi = pers.tile([32, 1], F32, tag="negpi")
            nc.vector.memset(negpi[:], float(-np.pi))

            # ---- persistent activations (bf16, [*, NLOC])
            h0 = pers.tile([P, NLOC], BF, tag="h0")
            cond = pers.tile([P, NLOC], BF, tag="cond")
            At = pers.tile([P, NLOC], BF, tag="At")
            T1 = pers.tile([P, NLOC], BF, tag="T1")
            T2 = pers.tile([4, NLOC], BF, tag="T2")

            # ---- node phase (per-chunk streaming)
            for i in range(NCH):
                s = slice(i * CH, (i + 1) * CH)
                xgc = wk.tile([6, CH], F32, tag="xgc")
                nc.sync.dma_start(out=xgc[:], in_=xg[:, s])
                pt = ps.tile([P, CH], F32, tag="pmm")
                nc.tensor.matmul(pt[:], wi1[:], xgc[:], start=True, stop=True)
                h0a = wk.tile([P, CH], BF, tag="h0a")
                nc.scalar.activation(out=h0a[:], in_=pt[:], func=ACT.Relu,
                                     bias=bi1[:])
                pt2 = ps.tile([P, CH], F32, tag="pmm")
                nc.tensor.matmul(pt2[:], wi2b[:], h0a[:], start=True, stop=True)
                nc.scalar.activation(out=h0[:, s], in_=pt2[:], func=ACT.Relu,
                                     bias=bi2[:])

                ohc = wk.tile([10, CH], F32, tag="ohc")
                nc.sync.dma_start(out=ohc[:], in_=cat_oh[:, s])
                pc = ps.tile([64, CH], F32, tag="pmm")
                nc.tensor.matmul(pc[:], embt[:], ohc[:], start=True, stop=True)
                nc.scalar.activation(out=cond[0:64, s], in_=pc[:], func=ACT.Relu)

                trc = wk.tile([1, CH], F32, tag="trc")
                nc.sync.dma_start(out=trc[:], in_=t_row[:, s])
                ptt = ps.tile([32, CH], F32, tag="pmm")
                nc.tensor.matmul(ptt[:], ones32[:], trc[:], start=True, stop=True)
                MAGIC = 12582912.0  # 1.5*2^23: f32 add/sub rounds to int
                tp = wk.tile([32, CH], F32, tag="tp")
                nc.vector.tensor_scalar(out=tp[:], in0=ptt[:], scalar1=wg2pi[:],
                                        scalar2=None, op0=ALU.mult)
                gfc = wk.tile([64, CH], BF, tag="gfc")
                for half, phase in ((0, 0.0), (1, 0.25)):
                    v = wk.tile([32, CH], F32, tag="yr")
                    if phase == 0.0:
                        nc.vector.tensor_scalar(out=v[:], in0=ptt[:],
                                                scalar1=wgc[:], scalar2=MAGIC,
                                                op0=ALU.mult, op1=ALU.add)
                    else:
                        nc.vector.tensor_scalar(out=v[:], in0=ptt[:],
                                                scalar1=wgc[:],
                                                scalar2=float(phase),
                                                op0=ALU.mult, op1=ALU.add)
                        nc.vector.tensor_scalar(out=v[:], in0=v[:],
                                                scalar1=MAGIC, scalar2=None,
                                                op0=ALU.add)
                    m = wk.tile([32, CH], F32, tag="mr")
                    nc.vector.tensor_scalar(out=m[:], in0=v[:], scalar1=MAGIC,
                                            scalar2=None, op0=ALU.subtract)
                    nc.vector.tensor_scalar(
                        out=m[:], in0=m[:], scalar1=float(-2 * np.pi),
                        scalar2=float(phase * 2 * np.pi),
                        op0=ALU.mult, op1=ALU.add)
                    r = wk.tile([32, CH], F32, tag="rr")
                    nc.vector.tensor_tensor(out=r[:], in0=tp[:], in1=m[:],
                                            op=ALU.add)
                    nc.scalar.activation(out=gfc[half * 32:(half + 1) * 32, :],
                                         in_=r[:], func=ACT.Sin)
                pg = ps.tile([64, CH], F32, tag="pmm")
                nc.tensor.matmul(pg[:], wsigb[:], gfc[:], start=True, stop=True)
                nc.scalar.activation(out=cond[64:128, s], in_=pg[:],
                                     func=ACT.Relu, bias=bsig[:])

            def make_AB(hsrc, wda, wdb, wbh, wbc, bA, agin):
                for i in range(NCH):
                    s = slice(i * CH, (i + 1) * CH)
                    pt = ps.tile([P, CH], F32, tag="pmm")
                    nc.tensor.matmul(pt[:], wda[:], hsrc[:, s], start=True,
                                     stop=False)
                    nc.tensor.matmul(pt[:], wdb[:], cond[:, s], start=False,
                                     stop=True)
                    nc.scalar.activation(out=At[:, s], in_=pt[:],
                                         func=ACT.Identity, bias=bA[:])
                for q in range(NT):
                    s = slice(q * P, (q + 1) * P)
                    pb = ps.tile([P, P], F32, tag="pbr")
                    nc.tensor.matmul(pb[:], hsrc[:, s], wbh[:], start=True,
                                     stop=False)
                    nc.tensor.matmul(pb[:], cond[:, s], wbc[:], start=False,
                                     stop=True)
                    rows = wk.tile([P, P], BF, tag="brows")
                    nc.scalar.activation(out=rows[:], in_=pb[:], func=ACT.Copy)
                    nc.sync.dma_start(out=agin[s, :], in_=rows[:])

            def edge_conv(bfull, wout, outp, T):
                off16 = 0
                for q in range(NT):
                    nb = int(NB[q])
                    ncols = (nb + 1) * P
                    g = gp.tile([P, 1, (NBMAX + 1) * P], BF, tag="G")
                    # Single SWDGE queue: the ring FIFO serializes the
                    # transpose-gather xbar streams (concurrent streams from
                    # different queues corrupt data via the shared per-engine
                    # xbar). The enlarged descriptor carveout lets desc-gen
                    # of gather k+1 proceed while gather k drains.
                    nc.gpsimd.dma_gather(
                        g[:, :, :ncols], bfull[cfg.BASE:, :],
                        idxt[:, off16:off16 + ncols // 16],
                        ncols, ncols, P, transpose=True,
                        single_packet=False, queue_num=0)
                    qs = slice(q * P, (q + 1) * P)
                    for j0 in range(0, nb, 4):
                        jc = min(4, nb - j0)
                        gsl = g[:, 0, j0 * P:(j0 + jc) * P]
                        zt = wk.tile([P, 4 * P], BF, tag="zt")
                        ab = At[:, qs].unsqueeze(1).broadcast_to((P, jc, P))
                        nc.vector.tensor_tensor(
                            out=zt[:, :jc * P].rearrange("p (j c) -> p j c", c=P),
                            in0=gsl.rearrange("p (j c) -> p j c", c=P),
                            in1=ab, op=ALU.add)
                        nc.scalar.activation(out=zt[:, :jc * P],
                                             in_=zt[:, :jc * P], func=ACT.Relu)
                        pu = ps2.tile([outp, 4 * P], F32, tag="pu")
                        nc.tensor.matmul(pu[:, :jc * P], wout[:], zt[:, :jc * P],
                                         start=True, stop=True)
                        red = wk.tile([outp, P], BF, tag="red")
                        nc.vector.tensor_reduce(
                            out=red[:], in_=pu[:, :jc * P].rearrange(
                                "p (j c) -> p c j", c=P),
                            axis=AX.X, op=ALU.max)
                        nc.vector.tensor_tensor(out=T[:, qs], in0=T[:, qs],
                                                in1=red[:], op=ALU.max)
                    off16 += ncols // 16

            # ---- conv1
            nc.vector.memset(T1[:], -1e30)
            make_AB(h0, w1da, w1db, w1bh, w1bc, b11t, agin1)
            nc.gpsimd.collective_compute(
                "AllGather", ALU.bypass,
                replica_groups=[list(range(cfg.NC))],
                ins=[agin1.ap().opt()], outs=[bfull1.ap().opt()])
            edge_conv(bfull1, w12b, P, T1)

            # ---- h1 = relu(T1 + b12) into h0 buffer; conv2
            for i in range(NCH):
                s = slice(i * CH, (i + 1) * CH)
                nc.scalar.activation(out=h0[:, s], in_=T1[:, s], func=ACT.Relu,
                                     bias=b12t[:])
            nc.vector.memset(T2[:], -1e30)
            make_AB(h0, w2da, w2db, w2bh, w2bc, b21t, agin2)
            nc.gpsimd.collective_compute(
                "AllGather", ALU.bypass,
                replica_groups=[list(range(cfg.NC))],
                ins=[agin2.ap().opt()], outs=[bfull2.ap().opt()])
            edge_conv(bfull2, w22b, 4, T2)

            # ---- epilogue per chunk: out = (T2 + b22) / (std + 1e-7)
            for i in range(NCH):
                s = slice(i * CH, (i + 1) * CH)
                trc = wk.tile([1, CH], F32, tag="trc")
                nc.sync.dma_start(out=trc[:], in_=t_row[:, s])
                sr = wk.tile([1, CH], F32, tag="sr")
                nc.scalar.activation(out=sr[:], in_=trc[:], func=ACT.Exp,
                                     scale=LN2S)
                nc.scalar.activation(out=sr[:], in_=sr[:], func=ACT.Sqrt,
                                     scale=float(1.0 / LN2S), bias=negl[:])
                nc.vector.tensor_scalar_add(out=sr[:], in0=sr[:], scalar1=1e-7)
                iv = wk.tile([1, CH], F32, tag="iv")
                nc.vector.reciprocal(out=iv[:], in_=sr[:])
                pt4 = ps.tile([4, CH], F32, tag="pmm")
                nc.tensor.matmul(pt4[:], ones4[:], iv[:], start=True, stop=True)
                osb = wk.tile([4, CH], F32, tag="osb")
                nc.scalar.activation(out=osb[:], in_=T2[:, s],
                                     func=ACT.Identity, bias=b22t[:])
                nc.vector.tensor_tensor(out=osb[:], in0=osb[:], in1=pt4[:],
                                        op=ALU.mult)
                nc.sync.dma_start(out=out[:, s], in_=osb[:])
    return nc


# ----------------------------------------------------------------- entry point

_CACHE = {}


def kernel(**inputs):
    """Self-contained Trainium2 kernel: takes FULL inputs, returns FULL [N,4]."""
    from concourse.bass_utils import run_bass_kernel_spmd

    cfg = Cfg(50000, 800000, base=32768, ch=448)
    edge_key = None
    lay = build_layout(cfg, np.asarray(inputs["edge_index"]))
    maps = shard_inputs(cfg, inputs, lay)
    nc = build_graph(cfg, lay["NB"], lay["nslots"])
    nc.compile()
    res = run_bass_kernel_spmd(nc, maps, list(range(cfg.NC)))
    outs = [res.results[c]["out"] for c in range(cfg.NC)]
    return unshard_output(cfg, outs, lay)

